# revision 8
# baseline (speedup 1.0000x reference)
"""MoE BERT head (soft routing) on 8 Trainium2 NeuronCores.

Reference computation (B=4096, H=1024, E=32, O=1024):
    gate = softmax(X @ gate_W + gate_b, axis=experts)            [B, E]
    h_e  = relu(LN(X @ W1[e] + b1[e]) * ln_g[e] + ln_b[e])       [B, H] per expert
    out  = sum_e gate[:, e] * (h_e @ W2[e] + b2[e])              [B, O]

Strategy: expert-parallel over 8 cores (4 experts/core), single pass.
The gate softmax is computed on the host (268 MFLOP of BLAS) and shipped
as a tiny per-core [B, 4] tensor; on device it folds into the LayerNorm
scale.  Batch is processed in column-chunks of 1024; per chunk each
expert's first GEMM runs bt-outer so LayerNorm applies straight out of
PSUM (no staging copy), activations are PE-transposed into [feature,
batch] layout (with relu fused into the copyback) and the second GEMM
accumulates ALL FOUR experts in one PSUM group (K-concatenation,
K=4096).  Each core returns one partial out.T [O, B]; the host sums
partials over cores and transposes.  The per-expert output bias enters
the total as gate @ b2, added on the host (exact).

All matmuls run in bfloat16 with fp32 PSUM accumulation; on TRN2 the
moving operand streams 2 bf16 columns/cycle so this is 2x the fp32r
rate.  LayerNorm statistics are computed in fp32 via bn_stats on the
PSUM tiles.
"""

import os
import sys
from contextlib import ExitStack

for _p in ("/opt/trn_rl_repo", "/root/.axon_site/_ro/trn_rl_repo"):
    if os.path.isdir(_p) and _p not in sys.path:
        sys.path.insert(0, _p)

import ml_dtypes
import numpy as np

import concourse.bass as bass
import concourse.mybir as mybir
import concourse.tile as tile
from concourse import bacc
from concourse.bass_utils import run_bass_kernel_spmd
from concourse.masks import make_identity

B, H, E, O = 4096, 1024, 32, 1024
LN_EPS = 1e-5
N_CORES = 8
E_PER_CORE = E // N_CORES            # 4
B_CHUNK = 1024
N_CHUNKS = B // B_CHUNK              # 4
P = 128
KT = H // P                          # 8 k-tiles over the hidden dim
BT = B_CHUNK // P                    # 8 batch tiles per chunk
NKE = E_PER_CORE * KT                # 32 k-tiles for the K-concat GEMM2
F32 = mybir.dt.float32
BF16 = mybir.dt.bfloat16

Relu = mybir.ActivationFunctionType.Relu
Sqrt = mybir.ActivationFunctionType.Sqrt
Alu = mybir.AluOpType
AxX = mybir.AxisListType.X

_CACHE = {}
_LAST_IN_MAPS = None

# scheduling knobs (pool depths / grouping)
_CFG = {
    "xtp": 16, "w1p": 32, "w2p": 2, "hscp": 8, "hstp": 5, "osbp": 4,
    "hps": 4, "tps": 2, "ops": 2,
}


class _K:
    """Holds program-build state (pools, dram handles, flags)."""


def _load_chunk_xt(k_, c0):
    nc = k_.nc
    xts = []
    for k in range(KT):
        t = k_.xtp.tile([P, B_CHUNK], BF16, tag="xt")
        nc.sync.dma_start(out=t[:], in_=k_.xt_d[k * P:(k + 1) * P, c0:c0 + B_CHUNK])
        xts.append(t)
    return xts


def _expert_gemm1_ln(k_, xts, gsel, e):
    """GEMM1 + LayerNorm (straight from PSUM) + gating fold + PE transpose.

    Returns hsT_e [P, KT, B_CHUNK] bf16 in [feature, batch] layout,
    already relu'd and scaled by the gating weight.
    """
    nc = k_.nc
    b1_bc = lng_bc = lnb_bc = None
    if k_.use_b1:
        b1_bc = k_.bcp.tile([P, H], F32, tag="b1bc")
        nc.gpsimd.dma_start(out=b1_bc[:], in_=k_.b1_d[e].partition_broadcast(P))
    if k_.use_lng:
        lng_bc = k_.bcp.tile([P, H], F32, tag="lngbc")
        nc.gpsimd.dma_start(out=lng_bc[:], in_=k_.lng_d[e].partition_broadcast(P))
    if k_.use_lnb:
        lnb_bc = k_.bcp.tile([P, H], F32, tag="lnbbc")
        nc.gpsimd.dma_start(out=lnb_bc[:], in_=k_.lnb_d[e].partition_broadcast(P))

    w1s = {}
    for dch in range(2):
        for k in range(KT):
            t = k_.w1p.tile([P, 512], BF16, tag="w1", name=f"w1_{k}_{dch}")
            nc.sync.dma_start(
                out=t[:],
                in_=k_.w1_d[e, k * P:(k + 1) * P, dch * 512:(dch + 1) * 512])
            w1s[(k, dch)] = t

    hsT_e = k_.hstp.tile([P, KT, B_CHUNK], BF16, tag="hsT")

    for bt in range(BT):
        # ---- GEMM1 for this batch tile: h[bt] = X[bt] @ W1[e]  (2 psum banks)
        hp = [k_.hps.tile([P, 512], F32, tag="hps", name=f"hps{i}") for i in range(2)]
        for dch in range(2):
            for k in range(KT):
                nc.tensor.matmul(
                    hp[dch][:],
                    xts[k][:, bt * P:(bt + 1) * P],
                    w1s[(k, dch)][:],
                    start=(k == 0), stop=(k == KT - 1))

        # ---- LayerNorm stats from PSUM
        stats = k_.smallp.tile([P, 2, 6], F32, tag="stats")
        if k_.use_b1:
            for dch in range(2):
                nc.vector.tensor_add(
                    hp[dch][:], hp[dch][:], b1_bc[:, dch * 512:(dch + 1) * 512])
        for dch in range(2):
            nc.vector.bn_stats(stats[:, dch, :], hp[dch][:])
        mv = k_.smallp.tile([P, 2], F32, tag="mv")
        nc.vector.bn_aggr(mv[:], stats[:])
        rg = k_.smallp.tile([P, 1], F32, tag="rg")
        nc.scalar.activation(rg[:], mv[:, 1:2], Sqrt, bias=k_.eps_t[:])
        nc.vector.reciprocal(rg[:], rg[:])
        nc.vector.tensor_mul(rg[:], rg[:], gsel[:, bt, e:e + 1])

        # ---- apply: h_sc = (h - mean) * (rstd * gate), PSUM -> SBUF bf16
        h_sc = k_.hscp.tile([P, H], BF16, tag="hsc")
        for dch in range(2):
            nc.vector.tensor_scalar(
                h_sc[:, dch * 512:(dch + 1) * 512], hp[dch][:],
                mv[:, 0:1], rg[:], op0=Alu.subtract, op1=Alu.mult)
        if k_.use_lng:
            nc.vector.tensor_mul(h_sc[:], h_sc[:], lng_bc[:])
        if k_.use_lnb:
            nc.vector.scalar_tensor_tensor(
                h_sc[:], lnb_bc[:], gsel[:, bt, e:e + 1], h_sc[:],
                op0=Alu.mult, op1=Alu.add)

        # ---- PE transpose -> relu -> [feature, batch] bf16
        for half in range(2):
            tp = k_.tps.tile([P, 4, P], BF16, tag="tps")
            for q in range(4):
                k = half * 4 + q
                nc.tensor.transpose(
                    tp[:, q, :],
                    h_sc[:, k * P:(k + 1) * P],
                    k_.ident[:])
            nc.scalar.activation(
                hsT_e[:, half * 4:(half + 1) * 4, bt * P:(bt + 1) * P],
                tp[:], Relu)
    return hsT_e


def _gemm2(k_, hsT, c0):
    """out.T += sum over all 4 experts: W2[e].T @ hsT[e] (PSUM K-concat)."""
    nc = k_.nc
    for ot in range(O // P):
        w2sb = k_.w2p.tile([P, NKE, P], BF16, tag="w2")
        nc.sync.dma_start(out=w2sb[:], in_=k_.w2_d[ot])
        for bs in range(B_CHUNK // 512):
            op_t = k_.ops.tile([P, 512], F32, tag="ops")
            for ke in range(NKE):
                nc.tensor.matmul(
                    op_t[:],
                    w2sb[:, ke, :],
                    hsT[ke // KT][:, ke % KT, bs * 512:(bs + 1) * 512],
                    start=(ke == 0), stop=(ke == NKE - 1))
            osb = k_.osbp.tile([P, 512], F32, tag="osb")
            nc.scalar.copy(osb[:], op_t[:])
            nc.sync.dma_start(
                out=k_.outp_d[ot * P:(ot + 1) * P,
                              c0 + bs * 512:c0 + (bs + 1) * 512],
                in_=osb[:])


def _build_program(use_b1, use_lng, use_lnb):
    nc = bacc.Bacc("TRN2", target_bir_lowering=False, debug=False,
                   num_devices=N_CORES)
    k_ = _K()
    k_.nc = nc
    k_.use_b1, k_.use_lng, k_.use_lnb = use_b1, use_lng, use_lnb

    k_.xt_d = nc.dram_tensor("xt", [H, B], BF16, kind="ExternalInput")
    k_.w1_d = nc.dram_tensor("w1", [E_PER_CORE, H, H], BF16, kind="ExternalInput")
    k_.w2_d = nc.dram_tensor("w2t", [O // P, P, NKE, P], BF16,
                             kind="ExternalInput")
    k_.gs_d = nc.dram_tensor("gs", [N_CHUNKS, P, BT, E_PER_CORE], F32,
                             kind="ExternalInput")
    k_.b1_d = (nc.dram_tensor("b1", [E_PER_CORE, H], F32, kind="ExternalInput")
               if use_b1 else None)
    k_.lng_d = (nc.dram_tensor("lng", [E_PER_CORE, H], F32, kind="ExternalInput")
                if use_lng else None)
    k_.lnb_d = (nc.dram_tensor("lnb", [E_PER_CORE, H], F32, kind="ExternalInput")
                if use_lnb else None)
    k_.outp_d = nc.dram_tensor("outp", [O, B], F32, kind="ExternalOutput")

    with tile.TileContext(nc) as tc, ExitStack() as ctx:
        pool = lambda name, bufs, **kw: ctx.enter_context(
            tc.tile_pool(name=name, bufs=bufs, **kw))
        singles = pool("singles", 1)
        k_.xtp = pool("xtp", _CFG["xtp"])
        k_.w1p = pool("w1p", _CFG["w1p"])
        k_.w2p = pool("w2p", _CFG["w2p"])
        k_.hscp = pool("hscp", _CFG["hscp"])
        k_.hstp = pool("hstp", _CFG["hstp"])
        k_.osbp = pool("osbp", _CFG["osbp"])
        k_.smallp = pool("smallp", 24)
        k_.gselp = pool("gselp", 2)
        k_.bcp = pool("bcp", 1)
        k_.hps = pool("hps", _CFG["hps"], space="PSUM")
        k_.tps = pool("tps", _CFG["tps"], space="PSUM")
        k_.ops = pool("ops", _CFG["ops"], space="PSUM")

        ident_f32 = singles.tile([P, P], F32)
        make_identity(nc, ident_f32)
        k_.ident = singles.tile([P, P], BF16)
        nc.vector.tensor_copy(k_.ident[:], ident_f32[:])
        k_.eps_t = singles.tile([P, 1], F32)
        nc.vector.memset(k_.eps_t, LN_EPS)

        for ci in range(N_CHUNKS):
            c0 = ci * B_CHUNK
            xts = _load_chunk_xt(k_, c0)
            gsel = k_.gselp.tile([P, BT, E_PER_CORE], F32, tag="gsel")
            nc.sync.dma_start(out=gsel[:], in_=k_.gs_d[ci])
            hsT = []
            for e in range(E_PER_CORE):
                hsT.append(_expert_gemm1_ln(k_, xts, gsel, e))
            _gemm2(k_, hsT, c0)

    nc.compile()
    return nc


def kernel(pooled_output, gate_W, gate_b, W1, b1, ln_g, ln_b, W2, b2):
    X = np.asarray(pooled_output, dtype=np.float32)
    gate_W = np.asarray(gate_W, dtype=np.float32)
    gate_b = np.asarray(gate_b, dtype=np.float32)
    W1 = np.asarray(W1, dtype=np.float32)
    b1 = np.asarray(b1, dtype=np.float32)
    ln_g = np.asarray(ln_g, dtype=np.float32)
    ln_b = np.asarray(ln_b, dtype=np.float32)
    W2 = np.asarray(W2, dtype=np.float32)
    b2 = np.asarray(b2, dtype=np.float32)

    use_b1 = bool(np.any(b1 != 0.0))
    use_lng = bool(np.any(ln_g != 1.0))
    use_lnb = bool(np.any(ln_b != 0.0))

    key = (use_b1, use_lng, use_lnb)
    if key not in _CACHE:
        _CACHE[key] = _build_program(*key)
    nc = _CACHE[key]

    # host-side gate softmax (exact, in fp32)
    gate = X @ gate_W + gate_b[None, :]
    gate -= gate.max(axis=1, keepdims=True)
    np.exp(gate, out=gate)
    gate /= gate.sum(axis=1, keepdims=True)

    BF = ml_dtypes.bfloat16
    XT = np.ascontiguousarray(X.T).astype(BF)  # [H, B]

    in_maps = []
    for c in range(N_CORES):
        own = list(range(E_PER_CORE * c, E_PER_CORE * (c + 1)))
        w1_c = np.ascontiguousarray(W1[own]).astype(BF)
        # W2 tiled as [o_tile, 128, (e, kd), 128]
        w2_c = W2[own].reshape(E_PER_CORE, KT, P, O // P, P)
        w2_c = np.ascontiguousarray(w2_c.transpose(3, 2, 0, 1, 4))
        w2_c = w2_c.reshape(O // P, P, NKE, P).astype(BF)
        # gate columns for this core's experts: [N_CHUNKS, 128, BT, E_PER_CORE]
        gs_c = np.ascontiguousarray(
            gate[:, own].reshape(N_CHUNKS, BT, P, E_PER_CORE)
            .transpose(0, 2, 1, 3))
        m = {
            "xt": XT,
            "w1": w1_c,
            "w2t": w2_c,
            "gs": gs_c,
        }
        if use_b1:
            m["b1"] = np.ascontiguousarray(b1[own])
        if use_lng:
            m["lng"] = np.ascontiguousarray(ln_g[own])
        if use_lnb:
            m["lnb"] = np.ascontiguousarray(ln_b[own])
        in_maps.append(m)

    global _LAST_IN_MAPS
    _LAST_IN_MAPS = in_maps
    res = run_bass_kernel_spmd(nc, in_maps, core_ids=list(range(N_CORES)))

    acc = res.results[0]["outp"].astype(np.float32)
    for c in range(1, N_CORES):
        acc += res.results[c]["outp"]
    out = np.ascontiguousarray(acc.T)
    if np.any(b2 != 0.0):
        # per-expert output bias enters as gate @ b2 ([B,E] @ [E,O])
        out += gate @ b2
    return np.ascontiguousarray(out, dtype=np.float32)


if __name__ == "__main__":
    rng = np.random.default_rng(0)
    s = 0.02
    inputs = {
        "pooled_output": rng.standard_normal((B, H), dtype=np.float32),
        "gate_W": rng.standard_normal((H, E), dtype=np.float32) * s,
        "gate_b": np.zeros((E,), np.float32),
        "W1": rng.standard_normal((E, H, H), dtype=np.float32) * s,
        "b1": np.zeros((E, H), np.float32),
        "ln_g": np.ones((E, H), np.float32),
        "ln_b": np.zeros((E, H), np.float32),
        "W2": rng.standard_normal((E, H, O), dtype=np.float32) * s,
        "b2": np.zeros((E, O), np.float32),
    }
    out = kernel(**inputs)
    print("out", out.shape, out.dtype, np.abs(out).max())
